# revision 5
# baseline (speedup 1.0000x reference)
"""Multi-head causal attention on 8 trn2 NeuronCores.

Reference semantics (B=2, S=2048, D=1024, H=16, DK=DV=64):
    q = X @ WQ * 1/sqrt(DK); k = X @ WK; v = X @ WV          (per head)
    logits[i, j] = q[i] . k[j]   (i = key pos, j = query pos, causal i <= j)
    P = softmax_i(logits); out[j] = (sum_i P[i,j] v[i]) @ WO + bO

Sharding: 2 batches x 16 heads = 32 bh-pairs -> 4 heads/core, batch b = core//4.
Each core computes attention for its heads plus the partial output projection
x_part @ WO[rows of its heads]; the host sums the 4 partials per batch
(all-reduce step of the row-sharded WO) and adds bO.

Device layout per core:
    XT  [D, S]       input transposed (d on partitions)
    QT/KT [hd=256, S] head-major transposed projections (d_head on partitions)
    V   [S, 4, 65]   natural layout + ones column (col 64) -> matmul row-sums
    scores^T psum [i=128, 2 heads, 512 j]  -> exp on ScalarE -> PT bf16
    x~ psum [j=128, 4 heads, 65]: accumulate PT.T @ V_aug over i-chunks;
       col 64 = softmax denominators -> reciprocal -> scale -> x_n bf16
    x_n -> PE transpose -> xT [256, S] -> out = xT.T @ WO_part -> DRAM f32
"""

import functools

import numpy as np
import ml_dtypes

import concourse.bass as bass
import concourse.mybir as mybir
import concourse.tile as tile
from concourse import bacc
from concourse.bass_utils import run_bass_kernel_spmd
from concourse.masks import make_identity

B, S, D, H = 2, 2048, 1024, 16
DK = DV = 64
NCORES = 8
GROUP = NCORES // B          # cores per batch
HG = H // GROUP              # heads per core = 4
HD = HG * DK                 # per-core head dims = 256
P = 128
KC = D // P                  # 8 contraction chunks over D
JB = 512                     # query-block width for score matmuls
NJB = S // JB                # 4
NIC = S // P                 # 16 key chunks
NJC = S // P                 # 16 query chunks
VW = DV + 1                  # value width + ones column

BF16 = mybir.dt.bfloat16
F32 = mybir.dt.float32
NPBF16 = ml_dtypes.bfloat16
Exp = mybir.ActivationFunctionType.Exp


def build_nc() -> bass.Bass:
    nc = bacc.Bacc()
    xt = nc.declare_dram_parameter("xt", [D, S], BF16, isOutput=False)
    wq = nc.declare_dram_parameter("wq", [D, HD], BF16, isOutput=False)
    wk = nc.declare_dram_parameter("wk", [D, HD], BF16, isOutput=False)
    wv = nc.declare_dram_parameter("wv", [D, HD], BF16, isOutput=False)
    wo = nc.declare_dram_parameter("wo", [HD, D], BF16, isOutput=False)
    tri = nc.declare_dram_parameter("tri", [P, P], BF16, isOutput=False)
    out = nc.declare_dram_parameter("out_part", [S, D], F32, isOutput=True)

    out_t = out.rearrange("(c p) o -> p c o", p=P)

    with tile.TileContext(nc) as tc:
        with (
            tc.tile_pool(name="const", bufs=1) as const_pool,
            tc.tile_pool(name="big", bufs=1) as big_pool,
            tc.tile_pool(name="pt", bufs=34) as pt_pool,
            tc.tile_pool(name="small", bufs=8) as small_pool,
            tc.tile_pool(name="osb", bufs=4) as osb_pool,
            tc.tile_pool(name="mmps", bufs=2, space="PSUM") as mm_psum,
            tc.tile_pool(name="sps", bufs=2, space="PSUM") as s_psum,
            tc.tile_pool(name="avps", bufs=2, space="PSUM") as av_psum,
        ):
            ident = const_pool.tile([P, P], BF16)
            make_identity(nc, ident)
            tri_sb = const_pool.tile([P, P], BF16)
            nc.sync.dma_start(tri_sb, tri[:, :])

            xt_sb = big_pool.tile([P, KC, S], BF16, name="xt_sb")
            nc.sync.dma_start(xt_sb, xt.rearrange("(kc p) i -> p kc i", p=P))
            w_sbs = {}
            for name, w in (("wq", wq), ("wk", wk), ("wv", wv)):
                w_sbs[name] = big_pool.tile([P, KC, HD], BF16, name=f"{name}_sb")
                nc.sync.dma_start(w_sbs[name], w.rearrange("(kc p) m -> p kc m", p=P))
            wo_sb = big_pool.tile([P, HD // P, D], BF16, name="wo_sb")
            nc.sync.dma_start(wo_sb, wo.rearrange("(hc p) o -> p hc o", p=P))

            # --- Q^T, K^T projections: [hd, S] with head dims on partitions ---
            qt_sb = big_pool.tile([P, HD // P, S], BF16, name="qt_sb")
            kt_sb = big_pool.tile([P, HD // P, S], BF16, name="kt_sb")
            for w_sb, t_sb in ((w_sbs["wq"], qt_sb), (w_sbs["wk"], kt_sb)):
                for mc in range(HD // P):
                    for nb in range(NJB):
                        ps = mm_psum.tile([P, JB], F32, tag="mmps")
                        for kc in range(KC):
                            nc.tensor.matmul(
                                ps,
                                lhsT=w_sb[:, kc, mc * P : (mc + 1) * P],
                                rhs=xt_sb[:, kc, nb * JB : (nb + 1) * JB],
                                start=(kc == 0),
                                stop=(kc == KC - 1),
                            )
                        nc.any.tensor_copy(t_sb[:, mc, nb * JB : (nb + 1) * JB], ps)

            # --- V in natural layout [i, h, v] plus the ones column ---
            v_sb = big_pool.tile([P, NIC, HG, VW], BF16, name="v_sb")
            nc.vector.memset(v_sb[:, :, :, DV : DV + 1], 1.0)
            for ic in range(NIC):
                ps = mm_psum.tile([P, JB], F32, tag="mmps")
                for kc in range(KC):
                    nc.tensor.matmul(
                        ps[:, :HD],
                        lhsT=xt_sb[:, kc, ic * P : (ic + 1) * P],
                        rhs=w_sbs["wv"][:, kc, :],
                        start=(kc == 0),
                        stop=(kc == KC - 1),
                    )
                nc.any.tensor_copy(
                    v_sb[:, ic, :, 0:DV],
                    ps[:, :HD].rearrange("p (h v) -> p h v", v=DV),
                )

            xT_sb = big_pool.tile([P, HD // P, S], BF16, name="xT_sb")

            # --- attention, one 512-wide query block at a time ---
            for jb in range(NJB):
                nib = 4 * jb + 4  # causal: key chunks 0 .. 4*jb+3
                pt_tiles = {}
                for ib in range(nib):
                    for hp in range(HG // 2):  # pack 2 heads per psum tile
                        sps = s_psum.tile([P, 2, JB], F32, tag="sps")
                        for hh in range(2):
                            h = 2 * hp + hh
                            base = DK * (h % 2)
                            hc = h // 2
                            nc.tensor.matmul(
                                sps[:, hh, :],
                                lhsT=qt_sb[base : base + DK, hc, ib * P : (ib + 1) * P],
                                rhs=kt_sb[base : base + DK, hc, jb * JB : (jb + 1) * JB],
                                start=True,
                                stop=True,
                            )
                        pt = pt_pool.tile([P, 2, JB], BF16, tag="pt")
                        nc.scalar.activation(pt, sps, Exp)
                        pt_tiles[ib, hp] = pt

                for jj in range(4):
                    jc = 4 * jb + jj
                    xps = av_psum.tile([P, HG, VW], F32, tag="avps")
                    for h in range(HG):
                        hp, hh = divmod(h, 2)
                        for ib in range(jc + 1):
                            lhsT = pt_tiles[ib, hp][:, hh, jj * P : (jj + 1) * P]
                            if ib == jc:  # diagonal: zero the i > j half
                                ptd = small_pool.tile([P, P], BF16, tag="ptd")
                                nc.vector.tensor_mul(ptd, lhsT, tri_sb)
                                lhsT = ptd
                            nc.tensor.matmul(
                                xps[:, h, :],
                                lhsT=lhsT,
                                rhs=v_sb[:, ib, h, :],
                                start=(ib == 0),
                                stop=(ib == jc),
                            )
                    recip = small_pool.tile([P, HG], F32, tag="recip")
                    nc.vector.reciprocal(recip, xps[:, :, DV])
                    xn = small_pool.tile([P, HG, DV], BF16, tag="xn")
                    nc.vector.tensor_tensor(
                        xn,
                        xps[:, :, 0:DV],
                        recip[:, :, None].to_broadcast([P, HG, DV]),
                        mybir.AluOpType.mult,
                    )
                    xn_flat = xn.rearrange("p h v -> p (h v)")
                    for vc in range(HD // P):
                        tps = av_psum.tile([P, P], BF16, tag="avps")
                        nc.tensor.transpose(tps, xn_flat[:, vc * P : (vc + 1) * P], ident)
                        nc.any.tensor_copy(xT_sb[:, vc, jc * P : (jc + 1) * P], tps)

                    # partial output projection for this query chunk
                    for oc in range(D // JB):
                        ops = mm_psum.tile([P, JB], F32, tag="mmps")
                        for hc in range(HD // P):
                            nc.tensor.matmul(
                                ops,
                                lhsT=xT_sb[:, hc, jc * P : (jc + 1) * P],
                                rhs=wo_sb[:, hc, oc * JB : (oc + 1) * JB],
                                start=(hc == 0),
                                stop=(hc == HD // P - 1),
                            )
                        osb = osb_pool.tile([P, JB], F32, tag="osb")
                        nc.any.tensor_copy(osb, ops)
                        nc.sync.dma_start(out_t[:, jc, oc * JB : (oc + 1) * JB], osb)
    nc.compile()
    return nc


@functools.lru_cache(maxsize=1)
def _cached_nc() -> bass.Bass:
    return build_nc()


def make_in_maps(inputs, mask, WQ, WK, WV, WO, bO):
    scale = np.float32(1.0 / np.sqrt(DK))
    wq2 = np.ascontiguousarray((WQ.reshape(D, D) * scale).astype(NPBF16))
    wk2 = np.ascontiguousarray(WK.reshape(D, D).astype(NPBF16))
    wv2 = np.ascontiguousarray(WV.reshape(D, D).astype(NPBF16))
    wo2 = np.ascontiguousarray(WO.astype(NPBF16))
    tri = np.triu(np.ones((P, P), np.float32)).astype(NPBF16)
    xts = [
        np.ascontiguousarray(np.asarray(inputs[b]).T.astype(NPBF16)) for b in range(B)
    ]
    in_maps = []
    for c in range(NCORES):
        b, hg = divmod(c, GROUP)
        cols = slice(hg * HD, (hg + 1) * HD)
        in_maps.append(
            {
                "xt": xts[b],
                "wq": np.ascontiguousarray(wq2[:, cols]),
                "wk": np.ascontiguousarray(wk2[:, cols]),
                "wv": np.ascontiguousarray(wv2[:, cols]),
                "wo": np.ascontiguousarray(wo2[cols, :]),
                "tri": tri,
            }
        )
    return in_maps


def combine(results, bO):
    parts = [r["out_part"] for r in results]
    out = np.empty((B, S, D), np.float32)
    for b in range(B):
        acc = parts[b * GROUP].astype(np.float32).copy()
        for g in range(1, GROUP):
            acc += parts[b * GROUP + g]
        out[b] = acc + np.asarray(bO, np.float32)[None, :]
    return out


def kernel(**inputs) -> np.ndarray:
    nc = _cached_nc()
    in_maps = make_in_maps(**inputs)
    res = run_bass_kernel_spmd(nc, in_maps, core_ids=list(range(NCORES)))
    return combine(res.results, inputs["bO"])
